# revision 16
# baseline (speedup 1.0000x reference)
"""Multi-head self-attention (B=2, N=2048, C=1024, H=16) on 8 trn2 NeuronCores.

Sharding: core = b * 4 + g  (data parallel over batch B=2, tensor parallel
over 4 head-groups of 4 heads each).  Each core computes its head-group's
QKV projections, attention, and a partial output projection; the host sums
the 4 partials per batch (the "all-reduce") and adds the bias.

On-chip layout is fully "feature-on-partition" (transposed): the kernel
consumes x^T and produces out^T, so every matmul contracts along the
partition dim with no on-chip transposes.  Softmax runs along the key dim
which lives on partitions: the row-sum is obtained by augmenting V with 64
columns of ones (PE computes sum(exp(S)) replicated across 64 partitions),
and exp() never needs the max-subtraction because scores are O(6) here.

All matmuls run in fp32r (TF32-like, full PE speed for moving dim >= 256);
fp32r operands must be produced by a compute op that rounds to fp32r, so
DMA'd fp32 data passes through one DVE convert-copy.
"""

import sys

for _p in ("/opt/trn_rl_repo",):
    if _p not in sys.path:
        sys.path.append(_p)

import numpy as np

import concourse.bass as bass
import concourse.mybir as mybir
import concourse.tile as tile
from concourse import bacc
from concourse.bass_utils import run_bass_kernel_spmd

B, N, C = 2, 2048, 1024
H = 16
HS = C // H  # 64
G = 4  # head groups (tensor-parallel factor)
HPG = H // G  # heads per group = 4
GC = HPG * HS  # channels per group = 256
SCALE = HS**-0.5
P = 128
F32 = mybir.dt.float32
F32R = mybir.dt.float32r

_CACHED = {}


def build_bass(loop_n=1):
    nc = bacc.Bacc("TRN2", target_bir_lowering=False, debug=False)
    xT = nc.dram_tensor("xT", (C, N), F32, kind="ExternalInput").ap()
    wqT = nc.dram_tensor("wqT", (C, GC), F32, kind="ExternalInput").ap()
    wkT = nc.dram_tensor("wkT", (C, GC), F32, kind="ExternalInput").ap()
    wvT = nc.dram_tensor("wvT", (C, GC), F32, kind="ExternalInput").ap()
    woT = nc.dram_tensor("woT", (GC, C), F32, kind="ExternalInput").ap()
    outT = nc.dram_tensor("outT", (C, N), F32, kind="ExternalOutput").ap()

    KC = C // P  # 8 contraction chunks for the qkv projection
    MC = N // P  # 16 sequence chunks
    QC = GC // P  # 2 chunks of group channels

    with tile.TileContext(nc) as tc:
        import contextlib

        ctx = contextlib.ExitStack()
        with ctx:
            # ---- persistent sbuf pools -------------------------------------
            wpool = ctx.enter_context(tc.tile_pool(name="wpool", bufs=1))
            mpool = ctx.enter_context(tc.tile_pool(name="mpool", bufs=1))
            psum = ctx.enter_context(tc.tile_pool(name="psum", bufs=1, space="PSUM"))

            stage = ctx.enter_context(tc.tile_pool(name="stage", bufs=2))
            opool = ctx.enter_context(tc.tile_pool(name="opool", bufs=2))

            # ---- persistent tiles ------------------------------------------
            xr = mpool.tile([P, KC, N], F32R)  # x^T, rounded
            wqr = wpool.tile([P, KC, GC], F32R)
            wkr = wpool.tile([P, KC, GC], F32R)
            wvr = wpool.tile([P, KC, GC], F32R)
            wor = wpool.tile([P, QC, C], F32R)
            qr = mpool.tile([P, QC, N], F32R)  # Q^T for the group
            kr = mpool.tile([P, QC, N], F32R)  # K^T
            # va blocks: even heads [V | ones], odd heads [ones | V] so that
            # the attention output lands on the partition half matching the
            # head's slot in `an` (channels of chunk c = head 2c then 2c+1).
            va = mpool.tile([P, MC, HPG, P], F32R)
            ones_f = mpool.tile([P, 2, HS], F32)
            an = mpool.tile([P, QC, N], F32R)  # normalized attn^T

            # ---- stage + convert inputs to fp32r ---------------------------
            def stage_all():
                SW = 512

                def stage_in(src2d, dst2d, width, c):
                    for j in range(max(1, width // SW)):
                        w = min(SW, width)
                        st = stage.tile([P, SW], F32, tag="st", name="st")
                        nc.sync.dma_start(
                            out=st[:, :w], in_=src2d[c * P : (c + 1) * P, j * SW : j * SW + w]
                        )
                        nc.vector.tensor_copy(dst2d[:, c, j * SW : j * SW + w], st[:, :w])

                for c in range(KC):
                    stage_in(xT, xr, N, c)
                for w_dram, w_r in ((wqT, wqr), (wkT, wkr), (wvT, wvr)):
                    for c in range(KC):
                        stage_in(w_dram, w_r, GC, c)
                for c in range(QC):
                    stage_in(woT, wor, C, c)

            # ---- phase B: QKV projections ----------------------------------
            def qk_proj(w_r, dst, mch):
                for nch in range(4):
                    acc = psum.tile([P, 512], F32, tag="pb", bufs=2, name="acc")
                    for k in range(KC):
                        nc.tensor.matmul(
                            acc[:],
                            w_r[:, k, mch * P : (mch + 1) * P],
                            xr[:, k, nch * 512 : (nch + 1) * 512],
                            start=(k == 0),
                            stop=(k == KC - 1),
                        )
                    nc.vector.tensor_copy(dst[:, mch, nch * 512 : (nch + 1) * 512], acc[:])

            def v_proj():
                for m in range(MC):
                    vacc = psum.tile([P, GC], F32, tag="pb", bufs=2, name="vacc")
                    for k in range(KC):
                        nc.tensor.matmul(
                            vacc[:],
                            xr[:, k, m * P : (m + 1) * P],
                            wvr[:, k, :],
                            start=(k == 0),
                            stop=(k == KC - 1),
                        )
                    vh = vacc.rearrange("p (h e) -> p h e", h=HPG)
                    nc.vector.tensor_copy(va[:, m, 0::2, 0:HS], vh[:, 0::2, :])
                    nc.vector.tensor_copy(va[:, m, 0::2, HS:P], ones_f[:])
                    nc.vector.tensor_copy(va[:, m, 1::2, 0:HS], ones_f[:])
                    nc.vector.tensor_copy(va[:, m, 1::2, HS:P], vh[:, 1::2, :])

            # ---- phase C: attention for a head pair (2c, 2c+1) -------------
            # The two heads' score matmuls have K=64 stationaries at base
            # partitions 0 and 64 -> distinct PE row-groups; emitting them
            # back-to-back lets the hardware overlap them (~2x on the score
            # phase).  One exp covers both heads' 512-wide P tiles.
            def attention_pair(hp):
                h0, h1 = 2 * hp, 2 * hp + 1
                for q in range(4):  # query quarters of 512
                    qsl = slice(q * 512, (q + 1) * 512)
                    att0 = psum.tile([P, 512], F32, tag="att0", bufs=1, name="att0")
                    att1 = psum.tile([P, 512], F32, tag="att1", bufs=1, name="att1")
                    for m in range(MC):
                        s = psum.tile([P, 1024], F32, tag="s", bufs=2, name="s")
                        for par, off in ((0, 0), (1, 64)):
                            nc.tensor.matmul(
                                s[:, par * 512 : (par + 1) * 512],
                                kr[off : off + 64, hp, m * P : (m + 1) * P],
                                qr[off : off + 64, hp, qsl],
                                start=True,
                                stop=True,
                            )
                        p_sb = mpool.tile([P, 1024], F32R, tag="p_sb", bufs=2, name="p_sb")
                        nc.scalar.activation(
                            p_sb[:], s[:], mybir.ActivationFunctionType.Exp, scale=SCALE
                        )
                        for par, att in ((0, att0), (1, att1)):
                            nc.tensor.matmul(
                                att[:],
                                va[:, m, 2 * hp + par, :],
                                p_sb[:, par * 512 : (par + 1) * 512],
                                start=(m == 0),
                                stop=(m == MC - 1),
                            )
                    # normalize.  Even head: attn rows 0:64, rowsum 64:128;
                    # odd head: flipped (va block order).  Custom recip uop
                    # only works at base partition 0; cross-partition moves
                    # go through SBUF->SBUF DMA.
                    au0 = mpool.tile([P, 512], F32, tag="au", bufs=2, name="au0")
                    au1 = mpool.tile([P, 512], F32, tag="au", bufs=2, name="au1")
                    rr0 = mpool.tile([P, 512], F32, tag="rr", bufs=2, name="rr0")
                    rr1 = mpool.tile([P, 512], F32, tag="rr", bufs=2, name="rr1")
                    nc.vector.tensor_copy(au0[:], att0[:])
                    nc.vector.tensor_copy(au1[:], att1[:])
                    nc.sync.dma_start(out=rr0[0:64, :], in_=au0[64:128, :])
                    nc.vector.reciprocal_approx_fast(rr0[0:64, :], rr0[0:64, :])
                    nc.vector.tensor_mul(
                        an[0:64, hp, qsl], au0[0:64, :], rr0[0:64, :]
                    )
                    nc.vector.reciprocal_approx_fast(rr1[0:64, :], au1[0:64, :])
                    nc.sync.dma_start(out=rr1[64:128, :], in_=rr1[0:64, :])
                    nc.vector.tensor_mul(
                        an[64:128, hp, qsl], au1[64:128, :], rr1[64:128, :]
                    )

            # ---- phase E: output projection --------------------------------
            def out_proj():
                for och in range(C // P):
                    for nch in range(4):
                        o_ps = psum.tile([P, 512], F32, tag="pb", bufs=2, name="o_ps")
                        for c in range(QC):
                            nc.tensor.matmul(
                                o_ps[:],
                                wor[:, c, och * P : (och + 1) * P],
                                an[:, c, nch * 512 : (nch + 1) * 512],
                                start=(c == 0),
                                stop=(c == QC - 1),
                            )
                        o_sb = opool.tile([P, 512], F32, name="o_sb")
                        nc.vector.tensor_copy(o_sb[:], o_ps[:])
                        nc.sync.dma_start(
                            out=outT[och * P : (och + 1) * P, nch * 512 : (nch + 1) * 512],
                            in_=o_sb,
                        )

            # ---- body: emission order enables PE/ACT overlap ---------------
            def body():
                nc.vector.memset(ones_f, 1.0)
                stage_all()
                qk_proj(wkr, kr, 0)
                qk_proj(wqr, qr, 0)
                v_proj()
                attention_pair(0)
                qk_proj(wkr, kr, 1)
                qk_proj(wqr, qr, 1)
                attention_pair(1)
                out_proj()

            if loop_n > 1:
                ET = mybir.EngineType
                with tc.For_i(
                    0,
                    loop_n,
                    1,
                    hint_engines=(ET.PE, ET.Activation, ET.DVE, ET.SP),
                ):
                    body()
            else:
                body()

    nc.compile()
    return nc


def shard_inputs(x, w_qkv, w_out):
    """Host-side shard prep. Returns in_maps for cores 0..7 (core = b*4+g)."""
    # w_qkv row d = c_idx*3 + t  (t: 0=q, 1=k, 2=v)  [stride-3 interleave]
    wr = np.ascontiguousarray(w_qkv.reshape(C, 3, C))
    in_maps = []
    for b in range(B):
        xTb = np.ascontiguousarray(x[b].T)
        for g in range(G):
            sl = slice(g * GC, (g + 1) * GC)
            in_maps.append(
                {
                    "xT": xTb,
                    "wqT": np.ascontiguousarray(wr[sl, 0, :].T),
                    "wkT": np.ascontiguousarray(wr[sl, 1, :].T),
                    "wvT": np.ascontiguousarray(wr[sl, 2, :].T),
                    "woT": np.ascontiguousarray(w_out[:, sl].T),
                }
            )
    return in_maps


def kernel(x, w_qkv, w_out, b_out):
    x = np.asarray(x, dtype=np.float32)
    w_qkv = np.asarray(w_qkv, dtype=np.float32)
    w_out = np.asarray(w_out, dtype=np.float32)
    b_out = np.asarray(b_out, dtype=np.float32)

    if "nc" not in _CACHED:
        _CACHED["nc"] = build_bass()
    nc = _CACHED["nc"]

    in_maps = shard_inputs(x, w_qkv, w_out)
    res = run_bass_kernel_spmd(nc, in_maps, core_ids=list(range(8)))

    out = np.empty((B, N, C), dtype=np.float32)
    for b in range(B):
        acc = res.results[b * G + 0]["outT"].astype(np.float32)
        for g in range(1, G):
            acc = acc + res.results[b * G + g]["outT"]
        out[b] = acc.T + b_out
    return out


if __name__ == "__main__":
    rng = np.random.default_rng(0)
    x = rng.standard_normal((B, N, C), dtype=np.float32)
    w_qkv = rng.standard_normal((3 * C, C), dtype=np.float32) * C**-0.5
    w_out = rng.standard_normal((C, C), dtype=np.float32) * C**-0.5
    b_out = np.zeros((C,), dtype=np.float32)
    got = kernel(x, w_qkv, w_out, b_out)
    print("kernel ran, output shape", got.shape)


# revision 25
# speedup vs baseline: 1.1959x; 1.1959x over previous
"""Multi-head self-attention (B=2, N=2048, C=1024, H=16) on 8 trn2 NeuronCores.

Sharding: core = b * 4 + g  (data parallel over batch B=2, tensor parallel
over 4 head-groups of 4 heads each).  Each core computes its head-group's
QKV projections, attention, and a partial output projection; the host sums
the 4 partials per batch (the "all-reduce") and adds the bias.

On-chip layout is fully "feature-on-partition" (transposed): the kernel
consumes x^T and produces out^T, so every matmul contracts along the
partition dim with no on-chip transposes.  Softmax runs along the key dim
which lives on partitions: the row-sum is obtained by augmenting V with 64
columns of ones (PE computes sum(exp(S)) replicated across 64 partitions),
and exp() never needs the max-subtraction because scores are O(6) here.

All matmuls run in fp32r (TF32-like, full PE speed for moving dim >= 256);
fp32r operands must be produced by a compute op that rounds to fp32r, so
DMA'd fp32 data passes through one DVE convert-copy.
"""

import sys

for _p in ("/opt/trn_rl_repo",):
    if _p not in sys.path:
        sys.path.append(_p)

import numpy as np

import concourse.bass as bass
import concourse.mybir as mybir
import concourse.tile as tile
from concourse import bacc
from concourse.bass_utils import run_bass_kernel_spmd

B, N, C = 2, 2048, 1024
H = 16
HS = C // H  # 64
G = 4  # head groups (tensor-parallel factor)
HPG = H // G  # heads per group = 4
GC = HPG * HS  # channels per group = 256
SCALE = HS**-0.5
P = 128
F32 = mybir.dt.float32
F32R = mybir.dt.float32r

_CACHED = {}


def build_bass(loop_n=1, stage_in_loop=True, parts=("qkv2", "att2", "out")):
    nc = bacc.Bacc("TRN2", target_bir_lowering=False, debug=False)
    xT = nc.dram_tensor("xT", (C, N), F32, kind="ExternalInput").ap()
    wqT = nc.dram_tensor("wqT", (C, GC), F32, kind="ExternalInput").ap()
    wkT = nc.dram_tensor("wkT", (C, GC), F32, kind="ExternalInput").ap()
    wvT = nc.dram_tensor("wvT", (C, GC), F32, kind="ExternalInput").ap()
    woT = nc.dram_tensor("woT", (GC, C), F32, kind="ExternalInput").ap()
    outT = nc.dram_tensor("outT", (C, N), F32, kind="ExternalOutput").ap()

    KC = C // P  # 8 contraction chunks for the qkv projection
    MC = N // P  # 16 sequence chunks
    QC = GC // P  # 2 chunks of group channels

    with tile.TileContext(nc) as tc:
        import contextlib

        ctx = contextlib.ExitStack()
        with ctx:
            # ---- persistent sbuf pools -------------------------------------
            wpool = ctx.enter_context(tc.tile_pool(name="wpool", bufs=1))
            mpool = ctx.enter_context(tc.tile_pool(name="mpool", bufs=1))
            psum = ctx.enter_context(tc.tile_pool(name="psum", bufs=1, space="PSUM"))

            stage = ctx.enter_context(tc.tile_pool(name="stage", bufs=2))
            opool = ctx.enter_context(tc.tile_pool(name="opool", bufs=2))

            # ---- persistent tiles ------------------------------------------
            xr = mpool.tile([P, KC, N], F32R)  # x^T, rounded
            wqr = wpool.tile([P, KC, GC], F32R)
            wkr = wpool.tile([P, KC, GC], F32R)
            wvr = wpool.tile([P, KC, GC], F32R)
            wor = wpool.tile([P, QC, C], F32R)
            qr = mpool.tile([P, QC, N], F32R)  # Q^T for the group
            kr = mpool.tile([P, QC, N], F32R)  # K^T
            # va blocks: even heads [V | ones], odd heads [ones | V] so that
            # the attention output lands on the partition half matching the
            # head's slot in `an` (channels of chunk c = head 2c then 2c+1).
            va = mpool.tile([P, MC, HPG, P], F32R)
            ones_f = mpool.tile([P, 2, HS], F32)
            an = mpool.tile([P, QC, N], F32R)  # normalized attn^T

            # ---- stage + convert inputs to fp32r ---------------------------
            # Alternate DMA issue between the SP (HWDGE) and GpSimd (SWDGE)
            # queues so input loads don't serialize on one DGE ring.
            def stage_all():
                SW = 1024
                qcycle = [0]

                def stage_in(src2d, dst2d, width, c):
                    for j in range(max(1, width // SW)):
                        w = min(SW, width)
                        st = stage.tile([P, SW], F32, tag="st", name="st")
                        eng = nc.sync if qcycle[0] % 2 == 0 else nc.gpsimd
                        qcycle[0] += 1
                        eng.dma_start(
                            out=st[:, :w], in_=src2d[c * P : (c + 1) * P, j * SW : j * SW + w]
                        )
                        nc.vector.tensor_copy(dst2d[:, c, j * SW : j * SW + w], st[:, :w])

                for c in range(KC):
                    stage_in(xT, xr, N, c)
                for w_dram, w_r in ((wqT, wqr), (wkT, wkr), (wvT, wvr)):
                    w3 = w_dram.rearrange("(c p) m -> p c m", p=P)
                    for hh in range(2):
                        st = stage.tile([P, SW], F32, tag="st", name="st")
                        st3 = st.rearrange("p (c m) -> p c m", m=GC)
                        eng = nc.sync if qcycle[0] % 2 == 0 else nc.gpsimd
                        qcycle[0] += 1
                        eng.dma_start(out=st3, in_=w3[:, hh * 4 : (hh + 1) * 4, :])
                        nc.vector.tensor_copy(w_r[:, hh * 4 : (hh + 1) * 4, :], st3)
                wo3 = woT.rearrange("(c p) o -> p c o", p=P)
                for c in range(QC):
                    st = stage.tile([P, SW], F32, tag="st", name="st")
                    eng = nc.sync if qcycle[0] % 2 == 0 else nc.gpsimd
                    qcycle[0] += 1
                    eng.dma_start(out=st, in_=wo3[:, c, :])
                    nc.vector.tensor_copy(wor[:, c, :], st[:])

            # ---- phase B: QKV projections ----------------------------------
            def qk_proj(w_r, dst, mch):
                for nch in range(4):
                    acc = psum.tile([P, 512], F32, tag="pb", bufs=2, name="acc")
                    for k in range(KC):
                        nc.tensor.matmul(
                            acc[:],
                            w_r[:, k, mch * P : (mch + 1) * P],
                            xr[:, k, nch * 512 : (nch + 1) * 512],
                            start=(k == 0),
                            stop=(k == KC - 1),
                        )
                    nc.vector.tensor_copy(dst[:, mch, nch * 512 : (nch + 1) * 512], acc[:])

            def v_proj():
                for m in range(MC):
                    vacc = psum.tile([P, GC], F32, tag="pb", bufs=2, name="vacc")
                    for k in range(KC):
                        nc.tensor.matmul(
                            vacc[:],
                            xr[:, k, m * P : (m + 1) * P],
                            wvr[:, k, :],
                            start=(k == 0),
                            stop=(k == KC - 1),
                        )
                    vh = vacc.rearrange("p (h e) -> p h e", h=HPG)
                    nc.vector.tensor_copy(va[:, m, 0::2, 0:HS], vh[:, 0::2, :])
                    nc.vector.tensor_copy(va[:, m, 0::2, HS:P], ones_f[:])
                    nc.vector.tensor_copy(va[:, m, 1::2, 0:HS], ones_f[:])
                    nc.vector.tensor_copy(va[:, m, 1::2, HS:P], vh[:, 1::2, :])

            # ---- phase C: attention for a head pair (2c, 2c+1) -------------
            # The two heads' score matmuls have K=64 stationaries at base
            # partitions 0 and 64 -> distinct PE row-groups; emitting them
            # back-to-back lets the hardware overlap them (~2x on the score
            # phase).  One exp covers both heads' 512-wide P tiles.
            def attention_pair(hp, after_q=None):
                h0, h1 = 2 * hp, 2 * hp + 1
                for q in range(4):  # query quarters of 512
                    qsl = slice(q * 512, (q + 1) * 512)
                    att0 = psum.tile([P, 512], F32, tag="att0", bufs=1, name="att0")
                    att1 = psum.tile([P, 512], F32, tag="att1", bufs=1, name="att1")
                    for m in range(MC):
                        s = psum.tile([P, 1024], F32, tag="s", bufs=2, name="s")
                        for par, off in ((0, 0), (1, 64)):
                            nc.tensor.matmul(
                                s[:, par * 512 : (par + 1) * 512],
                                kr[off : off + 64, hp, m * P : (m + 1) * P],
                                qr[off : off + 64, hp, qsl],
                                start=True,
                                stop=True,
                            )
                        p_sb = mpool.tile([P, 1024], F32R, tag="p_sb", bufs=2, name="p_sb")
                        nc.scalar.activation(
                            p_sb[:], s[:], mybir.ActivationFunctionType.Exp, scale=SCALE
                        )
                        for par, att in ((0, att0), (1, att1)):
                            nc.tensor.matmul(
                                att[:],
                                va[:, m, 2 * hp + par, :],
                                p_sb[:, par * 512 : (par + 1) * 512],
                                start=(m == 0),
                                stop=(m == MC - 1),
                            )
                    # normalize.  Even head: attn rows 0:64, rowsum 64:128;
                    # odd head: flipped (va block order).  Custom recip uop
                    # only works at base partition 0; cross-partition moves
                    # go through SBUF->SBUF DMA.
                    au0 = mpool.tile([P, 512], F32, tag="au", bufs=2, name="au0")
                    au1 = mpool.tile([P, 512], F32, tag="au", bufs=2, name="au1")
                    rr0 = mpool.tile([P, 512], F32, tag="rr", bufs=2, name="rr0")
                    rr1 = mpool.tile([P, 512], F32, tag="rr", bufs=2, name="rr1")
                    nc.vector.tensor_copy(au0[:], att0[:])
                    nc.vector.tensor_copy(au1[:], att1[:])
                    nc.sync.dma_start(out=rr0[0:64, :], in_=au0[64:128, :])
                    nc.vector.reciprocal_approx_fast(rr0[0:64, :], rr0[0:64, :])
                    nc.vector.tensor_mul(
                        an[0:64, hp, qsl], au0[0:64, :], rr0[0:64, :]
                    )
                    nc.vector.reciprocal_approx_fast(rr1[0:64, :], au1[0:64, :])
                    nc.sync.dma_start(out=rr1[64:128, :], in_=rr1[0:64, :])
                    nc.vector.tensor_mul(
                        an[64:128, hp, qsl], au1[64:128, :], rr1[64:128, :]
                    )
                    if after_q is not None:
                        after_q(q)

            # ---- phase E: output projection (one query quarter) ------------
            def out_proj_quarter(nch):
                for och in range(C // P):
                    o_ps = psum.tile([P, 512], F32, tag="pb", bufs=2, name="o_ps")
                    for c in range(QC):
                        nc.tensor.matmul(
                            o_ps[:],
                            wor[:, c, och * P : (och + 1) * P],
                            an[:, c, nch * 512 : (nch + 1) * 512],
                            start=(c == 0),
                            stop=(c == QC - 1),
                        )
                    o_sb = opool.tile([P, 512], F32, name="o_sb")
                    nc.vector.tensor_copy(o_sb[:], o_ps[:])
                    eng = nc.sync if och % 2 == 0 else nc.gpsimd
                    eng.dma_start(
                        out=outT[och * P : (och + 1) * P, nch * 512 : (nch + 1) * 512],
                        in_=o_sb,
                    )

            # ---- body: emission order enables PE/ACT overlap ---------------
            def body(staged):
                nc.vector.memset(ones_f, 1.0)
                if staged:
                    stage_all()
                qk_proj(wkr, kr, 0)
                qk_proj(wqr, qr, 0)
                v_proj()
                attention_pair(0)
                if "qkv2" in parts:
                    qk_proj(wkr, kr, 1)
                    qk_proj(wqr, qr, 1)
                after = out_proj_quarter if "out" in parts else None
                if "att2" in parts:
                    attention_pair(1, after_q=after)
                elif "out" in parts:
                    for q in range(4):
                        out_proj_quarter(q)

            if loop_n > 1:
                if not stage_in_loop:
                    stage_all()
                ET = mybir.EngineType
                with tc.For_i(
                    0,
                    loop_n,
                    1,
                    hint_engines=(ET.PE, ET.Activation, ET.DVE, ET.SP),
                ):
                    body(staged=stage_in_loop)
            else:
                body(staged=True)

    nc.compile()
    return nc


def shard_inputs(x, w_qkv, w_out):
    """Host-side shard prep. Returns in_maps for cores 0..7 (core = b*4+g)."""
    # w_qkv row d = c_idx*3 + t  (t: 0=q, 1=k, 2=v)  [stride-3 interleave]
    wr = np.ascontiguousarray(w_qkv.reshape(C, 3, C))
    in_maps = []
    for b in range(B):
        xTb = np.ascontiguousarray(x[b].T)
        for g in range(G):
            sl = slice(g * GC, (g + 1) * GC)
            in_maps.append(
                {
                    "xT": xTb,
                    "wqT": np.ascontiguousarray(wr[sl, 0, :].T),
                    "wkT": np.ascontiguousarray(wr[sl, 1, :].T),
                    "wvT": np.ascontiguousarray(wr[sl, 2, :].T),
                    "woT": np.ascontiguousarray(w_out[:, sl].T),
                }
            )
    return in_maps


def kernel(x, w_qkv, w_out, b_out):
    x = np.asarray(x, dtype=np.float32)
    w_qkv = np.asarray(w_qkv, dtype=np.float32)
    w_out = np.asarray(w_out, dtype=np.float32)
    b_out = np.asarray(b_out, dtype=np.float32)

    if "nc" not in _CACHED:
        _CACHED["nc"] = build_bass()
    nc = _CACHED["nc"]

    in_maps = shard_inputs(x, w_qkv, w_out)
    res = run_bass_kernel_spmd(nc, in_maps, core_ids=list(range(8)))

    out = np.empty((B, N, C), dtype=np.float32)
    for b in range(B):
        acc = res.results[b * G + 0]["outT"].astype(np.float32)
        for g in range(1, G):
            acc = acc + res.results[b * G + g]["outT"]
        out[b] = acc.T + b_out
    return out


if __name__ == "__main__":
    rng = np.random.default_rng(0)
    x = rng.standard_normal((B, N, C), dtype=np.float32)
    w_qkv = rng.standard_normal((3 * C, C), dtype=np.float32) * C**-0.5
    w_out = rng.standard_normal((C, C), dtype=np.float32) * C**-0.5
    b_out = np.zeros((C,), dtype=np.float32)
    got = kernel(x, w_qkv, w_out, b_out)
    print("kernel ran, output shape", got.shape)
